# revision 1
# baseline (speedup 1.0000x reference)
"""BatchTopK SAE Trainium2 kernel (8 NeuronCores, SPMD data-parallel).

Algorithm (per core c, batch rows 256c..256c+255):
  encode:  post.T[f, m] = relu(W_enc @ (x - b_dec).T + b_enc) via split-bf16x3
           GEMM (hi/lo decomposition, fp32 PSUM accumulate) -- matches fp32
           reference to ~1e-6 while running the PE at bf16 rate.
           Fused per f-tile: ACT computes relu + per-partition value sums
           (for the sigma estimate), DVE extracts top-16 per dictionary row
           (max8 + match_replace + max8) as threshold candidates.
  topk:    the global batch top-(K*B) reduces to a scalar threshold t* =
           (K*B)-th largest activation.  Each core brackets t* analytically
           from its sigma estimate ([lo0, hi0], verified margins), counts
           elements >= hi0 exactly, band-filters + compacts candidates to
           [128, 256], AllGathers all cores' candidates (+ sidecars), then
           every core runs an identical branch-free fp32 secant iteration on
           the gathered array until count(>= t) == K*B exactly.
  decode:  x_hat = (post * (post >= t*)) @ W_dec.T + b_dec with bf16 masked
           activations / weights (set-exact; value error ~0.2%).

Everything runs in ONE SPMD launch; host only reshapes inputs and concats
the per-core [256, 768] output slices.
"""

import numpy as np

ACT_DIM = 768
DICT = 16384
K = 64
BATCH = 2048
NCORES = 8
ROWS = BATCH // NCORES        # 256 batch rows per core
FT = DICT // 128              # 128 dictionary tiles
DT = ACT_DIM // 128           # 6 contraction tiles
MT = ROWS // 128              # 2 output row tiles
L1_W = FT * 16                # 2048 level-1 candidate cols
NSEG = 16                     # level-2 segments (128 wide each)
L2_W = NSEG * 16              # 256 level-2 candidate cols
PAY = L2_W + 8                # gather payload (sidecars in cols 256..258)
NLOC = DICT * ROWS            # 4194304 activations per core
CTARGET = float(K * BATCH)    # 131072
NSECANT = 11
import os as _os
ENCODE_MODE = _os.environ.get("SAE_ENCODE_MODE", "split3")  # split3 | fp32r

# Bracket constants: t* = sigma * z * (1 + model error); margins +-1.5%
# verified offline against two datasets (model error observed <= +0.42%).
_Z = 2.66007 * 1.002
A_LO = float(np.float32(_Z * 0.985))
A_HI = float(np.float32(_Z * 1.015))
SIG_SCALE = float(np.float32(np.sqrt(2.0 * np.pi) / NLOC))


def build_nc():
    from concourse import bass, bacc, mybir, tile, bass_isa

    dt = mybir.dt
    Alu = mybir.AluOpType
    nc = bacc.Bacc(num_devices=NCORES)

    # ---- DRAM I/O ----
    if ENCODE_MODE == "split3":
        xdt, wdt = dt.bfloat16, dt.bfloat16
    else:  # fp32r: same bytes as fp32, full-rate PE mode
        xdt, wdt = dt.float32r, dt.float32r
    xt_hi = nc.dram_tensor("xt_hi", [128, DT, ROWS], xdt, kind="ExternalInput")
    wenc_hi = nc.dram_tensor("wenc_hi", [FT, 128, DT, 128], wdt, kind="ExternalInput")
    if ENCODE_MODE == "split3":
        xt_lo = nc.dram_tensor("xt_lo", [128, DT, ROWS], xdt, kind="ExternalInput")
        wenc_lo = nc.dram_tensor("wenc_lo", [FT, 128, DT, 128], wdt, kind="ExternalInput")
    wdect = nc.dram_tensor("wdect", [FT, 128, ACT_DIM], dt.bfloat16, kind="ExternalInput")
    benc = nc.dram_tensor("benc", [128, FT], dt.float32, kind="ExternalInput")
    bdec_b = nc.dram_tensor("bdec_b", [128, ACT_DIM], dt.float32, kind="ExternalInput")
    xhat = nc.dram_tensor("xhat", [ROWS, ACT_DIM], dt.float32, kind="ExternalOutput")

    with tile.TileContext(nc) as tc:
        with (
            tc.tile_pool(name="persist", bufs=1) as P,
            tc.tile_pool(name="dram", bufs=1, space="DRAM") as D,
        ):
            post = P.tile([128, FT * ROWS], dt.float32, tag="post")
            l1 = P.tile([128, L1_W], dt.float32, tag="l1")
            sums = P.tile([128, FT], dt.float32, tag="sums")
            xh_s = P.tile([128, DT, ROWS], xdt, tag="xh")
            if ENCODE_MODE == "split3":
                xl_s = P.tile([128, DT, ROWS], xdt, tag="xl")
            benc_s = P.tile([128, FT], dt.float32, tag="benc")
            bdec_s = P.tile([128, ACT_DIM], dt.float32, tag="bdec")
            l2 = P.tile([128, PAY], dt.float32, tag="l2")
            gath = P.tile([128, NCORES, PAY], dt.float32, tag="gath")
            cscr_a = P.tile([128, L1_W], dt.float32, tag="cscr_a")
            cscr_b = P.tile([128, NCORES, L2_W], dt.float32, tag="cscr_b")
            g_in = D.tile([128, PAY], dt.float32)
            g_out = D.tile([NCORES, 128, PAY], dt.float32, addr_space="Shared")

            # scalar state tiles [128, 1]
            def sc(tag):
                return P.tile([128, 1], dt.float32, tag=tag, name=tag)

            sig = sc("sig"); lo0 = sc("lo0"); hi0 = sc("hi0"); chp = sc("chp")
            clp = sc("clp")
            lo = sc("lo"); hi = sc("hi"); clo = sc("clo"); chi = sc("chi")
            chg = sc("chg"); t = sc("t"); ct = sc("ct"); cp = sc("cp")
            pred = sc("pred"); npred = sc("npred"); tfin = sc("tfin")
            inv = sc("inv"); tmp1 = sc("tmp1"); tmp2 = sc("tmp2")
            found = sc("found"); ans = sc("ans")

            nc.sync.dma_start(out=xh_s[:], in_=xt_hi[:])
            if ENCODE_MODE == "split3":
                nc.sync.dma_start(out=xl_s[:], in_=xt_lo[:])
            nc.sync.dma_start(out=benc_s[:], in_=benc[:])
            nc.sync.dma_start(out=bdec_s[:], in_=bdec_b[:])

            # ================= encode =================
            with (
                tc.tile_pool(name="wenc", bufs=3) as WP,
                tc.tile_pool(name="epsum", bufs=4, space="PSUM") as EP,
                tc.tile_pool(name="escr", bufs=2) as ES,
            ):
                for ft in range(FT):
                    weh = WP.tile([128, DT, 128], wdt, tag="weh")
                    nc.sync.dma_start(out=weh[:], in_=wenc_hi[ft])
                    if ENCODE_MODE == "split3":
                        wel = WP.tile([128, DT, 128], wdt, tag="wel")
                        nc.sync.dma_start(out=wel[:], in_=wenc_lo[ft])
                        pairs = ((weh, xh_s), (weh, xl_s), (wel, xh_s))
                    else:
                        pairs = ((weh, xh_s),)
                    ps = EP.tile([128, ROWS], dt.float32, tag="eps")
                    n_mm = len(pairs) * DT
                    i = 0
                    for dtile in range(DT):
                        for wop, xop in pairs:
                            nc.tensor.matmul(
                                ps[:],
                                wop[:, dtile, :],
                                xop[:, dtile, :],
                                start=(i == 0),
                                stop=(i == n_mm - 1),
                            )
                            i += 1
                    pslice = post[:, ft * ROWS:(ft + 1) * ROWS]
                    nc.scalar.activation(
                        out=pslice,
                        in_=ps[:],
                        func=mybir.ActivationFunctionType.Relu,
                        bias=benc_s[:, ft:ft + 1],
                        scale=1.0,
                        accum_out=sums[:, ft:ft + 1],
                    )
                    # L1 candidates: top-16 of each dictionary row (256 cols)
                    scr = ES.tile([128, ROWS], dt.float32, tag="escr")
                    c0 = ft * 16
                    nc.vector.max(out=l1[:, c0:c0 + 8], in_=pslice)
                    nc.vector.match_replace(
                        out=scr[:], in_to_replace=l1[:, c0:c0 + 8],
                        in_values=pslice, imm_value=0.0,
                    )
                    nc.vector.max(out=l1[:, c0 + 8:c0 + 16], in_=scr[:])

            # ================= threshold =================
            # cross-partition sum+broadcast via PE: out[p] = sum_q in[q]
            ones_t = P.tile([128, 128], dt.float32, tag="ones_t")
            nc.vector.memset(ones_t[:], 1.0)
            with tc.tile_pool(name="rpsum", bufs=2, space="PSUM") as RP:

                def psum_reduce(in_ap, out_ap, scale=None, add_ap=None):
                    rps = RP.tile([128, 1], dt.float32, tag="rps", name="rps")
                    nc.tensor.matmul(rps[:], ones_t[:], in_ap, start=True, stop=True)
                    if scale is not None:
                        nc.vector.tensor_scalar_mul(out_ap, rps[:], scale)
                    elif add_ap is not None:
                        nc.vector.tensor_add(out_ap, rps[:], add_ap)
                    else:
                        nc.vector.tensor_copy(out_ap, rps[:])

                # sigma and bracket
                nc.vector.tensor_reduce(out=tmp1[:], in_=sums[:], axis=mybir.AxisListType.X, op=Alu.add)
                psum_reduce(tmp1[:], sig[:], scale=SIG_SCALE)
                nc.vector.tensor_scalar_mul(lo0[:], sig[:], A_LO)
                nc.vector.tensor_scalar_mul(hi0[:], sig[:], A_HI)

                # exact local counts >= hi0 / >= lo0 (all such elements are in l1)
                nc.vector.tensor_scalar(cscr_a[:], l1[:], hi0[:], None, op0=Alu.is_ge, op1=Alu.add, accum_out=chp[:])
                psum_reduce(chp[:], chp[:])
                nc.vector.tensor_scalar(cscr_a[:], l1[:], lo0[:], None, op0=Alu.is_ge, op1=Alu.add, accum_out=clp[:])
                psum_reduce(clp[:], clp[:])

                # band filter in place: keep values < hi0
                nc.vector.scalar_tensor_tensor(l1[:], l1[:], hi0[:], l1[:], op0=Alu.is_lt, op1=Alu.mult)

                # L2 compaction: top-16 per 128-wide segment
                with tc.tile_pool(name="l2scr", bufs=2) as LS:
                    for s in range(NSEG):
                        seg = l1[:, s * 128:(s + 1) * 128]
                        c0 = s * 16
                        sscr = LS.tile([128, 128], dt.float32, tag="sscr")
                        nc.vector.max(out=l2[:, c0:c0 + 8], in_=seg)
                        nc.vector.match_replace(
                            out=sscr[:], in_to_replace=l2[:, c0:c0 + 8],
                            in_values=seg, imm_value=0.0,
                        )
                        nc.vector.max(out=l2[:, c0 + 8:c0 + 16], in_=sscr[:])

                # sidecars
                nc.vector.tensor_copy(l2[:, L2_W:L2_W + 1], lo0[:])
                nc.vector.tensor_copy(l2[:, L2_W + 1:L2_W + 2], hi0[:])
                nc.vector.tensor_copy(l2[:, L2_W + 2:L2_W + 3], chp[:])
                nc.vector.tensor_copy(l2[:, L2_W + 3:L2_W + 4], clp[:])
                nc.vector.memset(l2[:, L2_W + 4:PAY], 0.0)

                # AllGather candidates
                nc.sync.dma_start(out=g_in[:], in_=l2[:])
                nc.gpsimd.collective_compute(
                    "AllGather",
                    Alu.bypass,
                    replica_groups=[list(range(NCORES))],
                    ins=[g_in.opt()],
                    outs=[g_out.opt()],
                )
                for c in range(NCORES):
                    nc.sync.dma_start(out=gath[:, c, :], in_=g_out[c])

                gv = gath[:, :, 0:L2_W]

                # global bracket / counts from sidecars
                nc.vector.tensor_reduce(out=lo[:], in_=gath[:, :, L2_W:L2_W + 1], axis=mybir.AxisListType.XY, op=Alu.max)
                nc.vector.tensor_reduce(out=hi[:], in_=gath[:, :, L2_W + 1:L2_W + 2], axis=mybir.AxisListType.XY, op=Alu.min)
                nc.vector.tensor_reduce(out=chg[:], in_=gath[:, :, L2_W + 2:L2_W + 3], axis=mybir.AxisListType.XY, op=Alu.add)
                nc.vector.tensor_reduce(out=clo[:], in_=gath[:, :, L2_W + 3:L2_W + 4], axis=mybir.AxisListType.XY, op=Alu.add)

                def count_ge(t_ap, out_ap):
                    nc.vector.tensor_scalar(cscr_b[:], gv, t_ap, None, op0=Alu.is_ge, op1=Alu.add, accum_out=cp[:])
                    psum_reduce(cp[:], out_ap, add_ap=chg[:])

                # exact count at the hi bracket; clo stays the (approximate)
                # sidecar sum -- it only steers the interpolation
                count_ge(hi[:], chi[:])

                tt = nc.vector.tensor_tensor
                nc.vector.memset(found[:], 0.0)
                nc.vector.memset(ans[:], 0.0)
                for it in range(NSECANT):
                    # t = hi + (C - chi) * (lo - hi) / max(clo - chi, 1)
                    tt(tmp1[:], clo[:], chi[:], op=Alu.subtract)
                    nc.vector.tensor_scalar_max(tmp1[:], tmp1[:], 1.0)
                    nc.vector.reciprocal(inv[:], tmp1[:])
                    nc.vector.tensor_scalar(tmp2[:], chi[:], -1.0, CTARGET, op0=Alu.mult, op1=Alu.add)
                    tt(tmp1[:], lo[:], hi[:], op=Alu.subtract)
                    tt(tmp1[:], tmp1[:], tmp2[:], op=Alu.mult)
                    tt(tmp1[:], tmp1[:], inv[:], op=Alu.mult)
                    tt(t[:], tmp1[:], hi[:], op=Alu.add)
                    # midpoint fallback if t fell outside (lo, hi)
                    tt(tmp1[:], t[:], lo[:], op=Alu.is_gt)
                    tt(tmp2[:], t[:], hi[:], op=Alu.is_lt)
                    tt(tmp1[:], tmp1[:], tmp2[:], op=Alu.mult)
                    tt(tmp2[:], lo[:], hi[:], op=Alu.add)
                    nc.vector.tensor_scalar_mul(tmp2[:], tmp2[:], 0.5)
                    tt(t[:], t[:], tmp2[:], op=Alu.subtract)
                    tt(t[:], tmp1[:], t[:], op=Alu.mult)
                    tt(t[:], t[:], tmp2[:], op=Alu.add)
                    count_ge(t[:], ct[:])
                    # latch the first exact hit
                    nc.vector.tensor_scalar(tmp1[:], ct[:], CTARGET, None, op0=Alu.is_equal)
                    nc.vector.tensor_scalar(tmp2[:], found[:], -1.0, 1.0, op0=Alu.mult, op1=Alu.add)
                    tt(tmp2[:], tmp1[:], tmp2[:], op=Alu.mult)
                    tt(found[:], found[:], tmp1[:], op=Alu.max)
                    tt(tmp1[:], t[:], ans[:], op=Alu.subtract)
                    tt(tmp1[:], tmp2[:], tmp1[:], op=Alu.mult)
                    tt(ans[:], ans[:], tmp1[:], op=Alu.add)
                    # bracket update
                    nc.vector.tensor_scalar(pred[:], ct[:], CTARGET, None, op0=Alu.is_ge)
                    nc.vector.tensor_scalar(npred[:], pred[:], -1.0, 1.0, op0=Alu.mult, op1=Alu.add)
                    tt(tmp1[:], t[:], lo[:], op=Alu.subtract)
                    tt(tmp1[:], pred[:], tmp1[:], op=Alu.mult)
                    tt(lo[:], lo[:], tmp1[:], op=Alu.add)
                    tt(tmp1[:], ct[:], clo[:], op=Alu.subtract)
                    tt(tmp1[:], pred[:], tmp1[:], op=Alu.mult)
                    tt(clo[:], clo[:], tmp1[:], op=Alu.add)
                    tt(tmp1[:], t[:], hi[:], op=Alu.subtract)
                    tt(tmp1[:], npred[:], tmp1[:], op=Alu.mult)
                    tt(hi[:], hi[:], tmp1[:], op=Alu.add)
                    tt(tmp1[:], ct[:], chi[:], op=Alu.subtract)
                    tt(tmp1[:], npred[:], tmp1[:], op=Alu.mult)
                    tt(chi[:], chi[:], tmp1[:], op=Alu.add)

                # tfin = found ? ans : lo
                tt(tmp1[:], found[:], ans[:], op=Alu.mult)
                nc.vector.tensor_scalar(tmp2[:], found[:], -1.0, 1.0, op0=Alu.mult, op1=Alu.add)
                tt(tmp2[:], tmp2[:], lo[:], op=Alu.mult)
                tt(tfin[:], tmp1[:], tmp2[:], op=Alu.add)

            # ================= decode =================
            with (
                tc.tile_pool(name="wdec", bufs=6) as WD,
                tc.tile_pool(name="dpsum", bufs=2, space="PSUM") as DP,
                tc.tile_pool(name="msk", bufs=10) as MS,
                tc.tile_pool(name="outs", bufs=2) as OS,
            ):
                HA = ACT_DIM // 2  # 384 — one matmul per PSUM bank
                pso = [
                    DP.tile([128, 2, 512], dt.float32, tag="dps", name=f"dps{mt}")
                    for mt in range(MT)
                ]
                for ft in range(FT):
                    pslice = post[:, ft * ROWS:(ft + 1) * ROWS]
                    mskt = MS.tile([128, ROWS], dt.bfloat16, tag="mskt")
                    nc.vector.scalar_tensor_tensor(
                        mskt[:], pslice, tfin[:], pslice, op0=Alu.is_ge, op1=Alu.mult
                    )
                    wd = WD.tile([128, ACT_DIM], dt.bfloat16, tag="wd")
                    nc.sync.dma_start(out=wd[:], in_=wdect[ft])
                    for mt in range(MT):
                        for h in range(2):
                            nc.tensor.matmul(
                                pso[mt][:, h, 0:HA],
                                mskt[:, mt * 128:(mt + 1) * 128],
                                wd[:, h * HA:(h + 1) * HA],
                                start=(ft == 0),
                                stop=(ft == FT - 1),
                            )
                for mt in range(MT):
                    outs = OS.tile([128, ACT_DIM], dt.float32, tag="outs")
                    for h in range(2):
                        nc.vector.tensor_add(
                            outs[:, h * HA:(h + 1) * HA],
                            pso[mt][:, h, 0:HA],
                            bdec_s[:, h * HA:(h + 1) * HA],
                        )
                    nc.sync.dma_start(out=xhat[mt * 128:(mt + 1) * 128, :], in_=outs[:])

    nc.finalize()
    return nc


def _prep_inputs(x, W_enc, b_enc, W_dec, b_dec):
    import ml_dtypes
    bf16 = ml_dtypes.bfloat16

    x0T = np.ascontiguousarray(
        (x.astype(np.float32) - b_dec.astype(np.float32)[None, :]).T
    )  # [768, 2048]
    WT = np.ascontiguousarray(W_enc.astype(np.float32).T)  # [768, 16384]

    def wlay(a):  # [768, 16384] -> [FT, 128(p=d), DT, 128(f)]
        return np.ascontiguousarray(
            a.reshape(DT, 128, FT, 128).transpose(2, 1, 0, 3)
        )

    if ENCODE_MODE == "split3":
        xh = x0T.astype(bf16)
        xl = (x0T - xh.astype(np.float32)).astype(bf16)
        Wh = WT.astype(bf16)
        Wl = (WT - Wh.astype(np.float32)).astype(bf16)
        WhL, WlL = wlay(Wh), wlay(Wl)
    else:
        xh, xl = x0T, None
        WhL, WlL = wlay(WT), None
    WdT = np.ascontiguousarray(W_dec.astype(np.float32).T).astype(bf16).reshape(FT, 128, ACT_DIM)
    bencL = np.ascontiguousarray(b_enc.astype(np.float32).reshape(FT, 128).T)
    bdecB = np.ascontiguousarray(
        np.broadcast_to(b_dec.astype(np.float32)[None, :], (128, ACT_DIM))
    )

    in_maps = []
    for c in range(NCORES):
        sl = slice(c * ROWS, (c + 1) * ROWS)
        m = {
            "xt_hi": np.ascontiguousarray(xh[:, sl].reshape(DT, 128, ROWS).transpose(1, 0, 2)),
            "wenc_hi": WhL,
            "wdect": WdT,
            "benc": bencL,
            "bdec_b": bdecB,
        }
        if ENCODE_MODE == "split3":
            m["xt_lo"] = np.ascontiguousarray(xl[:, sl].reshape(DT, 128, ROWS).transpose(1, 0, 2))
            m["wenc_lo"] = WlL
        in_maps.append(m)
    return in_maps


def _ensure_axon_hooks_shim():
    """concourse's trace path imports antenv.axon_hooks, which some images
    lack; install an equivalent module so tracing degrades (or works, when
    the ctypes hook is available) instead of crashing."""
    import sys, types
    try:
        import antenv.axon_hooks  # noqa: F401
        return
    except ImportError:
        pass
    m = types.ModuleType("antenv.axon_hooks")
    state = {"hook": None}
    m.set_axon_ntff_profile_hook = lambda h: state.__setitem__("hook", h)
    m.get_axon_ntff_profile_hook = lambda: state["hook"]
    sys.modules["antenv.axon_hooks"] = m
    try:
        from trn_agent_boot.trn_boot import _ntff_profile_via_ctypes
        hook = _ntff_profile_via_ctypes("/opt/axon/libaxon_pjrt.so")
        if hook is not None:
            m.set_axon_ntff_profile_hook(hook)
    except Exception:
        pass


def kernel(x, W_enc, b_enc, W_dec, b_dec):
    import os
    _ensure_axon_hooks_shim()
    from concourse import bass_utils
    from concourse.bass_utils import run_bass_kernel_spmd

    in_maps = _prep_inputs(x, W_enc, b_enc, W_dec, b_dec)
    nc = build_nc()
    res = None
    if os.environ.get("KERNEL_TRACE"):
        bass_utils.upload_artifacts = lambda d: ""  # no artifact bucket here
        try:
            res = run_bass_kernel_spmd(nc, in_maps, list(range(NCORES)), trace=True)
        except Exception as e:
            print(f"traced run failed ({type(e).__name__}: {e}); retrying untraced")
            res = None
    if res is None:
        res = run_bass_kernel_spmd(nc, in_maps, list(range(NCORES)))
    if res.exec_time_ns is not None:
        print(f"HW exec time: {res.exec_time_ns} ns")
    out = np.concatenate(
        [np.asarray(res.results[c]["xhat"], dtype=np.float32) for c in range(NCORES)],
        axis=0,
    )
    return out



# revision 4
# speedup vs baseline: 1.1716x; 1.1716x over previous
"""BatchTopK SAE Trainium2 kernel (8 NeuronCores, SPMD data-parallel).

Algorithm (per core c, batch rows 256c..256c+255):
  encode:  post.T[f, m] = relu(W_enc @ (x - b_dec).T + b_enc) via split-bf16x3
           GEMM (hi/lo decomposition, fp32 PSUM accumulate) -- matches fp32
           reference to ~1e-6 while running the PE at bf16 rate.
           Fused per f-tile: ACT computes relu + per-partition value sums
           (for the sigma estimate), DVE extracts top-8 per dictionary row
           (single max8) as threshold candidates.
           The threshold prep is folded INTO the encode loop: a provisional
           sigma from the first 32 f-tiles fixes the bracket [lo0, hi0];
           as each 8-tile segment of L1 completes it is counted (exact
           #>=hi0 / #>=lo0), band-filtered and compacted (top-8/segment),
           so when the last f-tile retires only the sidecars + AllGather
           remain.
  topk:    the global batch top-(K*B) reduces to a scalar threshold; each
           core AllGathers its 128 compacted candidates + sidecars, then
           runs an identical branch-free fp32 false-position iteration
           (6 rounds) on the gathered array.  The hi-side bracket is taken
           as the final threshold: count(>= hi) converges to within ~2 of
           K*B (verified in simulation), far inside the error budget.
  decode:  x_hat = (post * (post >= t)) @ W_dec.T + b_dec with bf16 masked
           activations / weights (value error ~0.2%).  Decoder weight tiles
           are prefetched during the threshold phase so the decode GEMM is
           PE-bound, not DMA-bound.

Everything runs in ONE SPMD launch; host only reshapes inputs and concats
the per-core [256, 768] output slices.
"""

import numpy as np

ACT_DIM = 768
DICT = 16384
K = 64
BATCH = 2048
NCORES = 8
ROWS = BATCH // NCORES        # 256 batch rows per core
FT = DICT // 128              # 128 dictionary tiles
DT = ACT_DIM // 128           # 6 contraction tiles
MT = ROWS // 128              # 2 output row tiles
L1_W = FT * 8                 # 1024 level-1 candidate cols (top-8/dict row)
NSEG = 16                     # level-2 segments (64 L1 cols = 8 f-tiles each)
SEGW = L1_W // NSEG           # 64
L2_W = NSEG * 8               # 128 level-2 candidate cols
PAY = L2_W + 8                # gather payload (sidecars in cols 128..131)
SIGT = 32                     # f-tiles used for the provisional sigma
CTARGET = float(K * BATCH)    # 131072
NSECANT = 6
NPREF = 24                    # decode weight tiles prefetched during topk

# Bracket constants: t* = sigma * z * (1 + model error); margins +-1.5%
# verified offline against two datasets (model error observed <= +0.42%,
# sigma sampling 3sd from 1M samples 0.17%).
_Z = 2.66007 * 1.002
A_LO = float(np.float32(_Z * 0.985))
A_HI = float(np.float32(_Z * 1.015))
SIG_SCALE = float(np.float32(np.sqrt(2.0 * np.pi) / (SIGT * 128 * ROWS)))


def build_nc():
    from concourse import bass, bacc, mybir, tile, bass_isa

    dt = mybir.dt
    Alu = mybir.AluOpType
    nc = bacc.Bacc(num_devices=NCORES)

    # ---- DRAM I/O ----
    xt_hi = nc.dram_tensor("xt_hi", [128, DT, ROWS], dt.bfloat16, kind="ExternalInput")
    xt_lo = nc.dram_tensor("xt_lo", [128, DT, ROWS], dt.bfloat16, kind="ExternalInput")
    wenc_hi = nc.dram_tensor("wenc_hi", [FT, 128, DT, 128], dt.bfloat16, kind="ExternalInput")
    wenc_lo = nc.dram_tensor("wenc_lo", [FT, 128, DT, 128], dt.bfloat16, kind="ExternalInput")
    wdect = nc.dram_tensor("wdect", [FT, 128, ACT_DIM], dt.bfloat16, kind="ExternalInput")
    benc = nc.dram_tensor("benc", [128, FT], dt.float32, kind="ExternalInput")
    bdec_b = nc.dram_tensor("bdec_b", [128, ACT_DIM], dt.float32, kind="ExternalInput")
    xhat = nc.dram_tensor("xhat", [ROWS, ACT_DIM], dt.float32, kind="ExternalOutput")

    with tile.TileContext(nc) as tc:
        with (
            tc.tile_pool(name="persist", bufs=1) as P,
            tc.tile_pool(name="dram", bufs=1, space="DRAM") as D,
        ):
            post = P.tile([128, FT * ROWS], dt.float32, tag="post")
            l1 = P.tile([128, L1_W], dt.float32, tag="l1")
            sums = P.tile([128, FT], dt.float32, tag="sums")
            xh_s = P.tile([128, DT, ROWS], dt.bfloat16, tag="xh")
            xl_s = P.tile([128, DT, ROWS], dt.bfloat16, tag="xl")
            benc_s = P.tile([128, FT], dt.float32, tag="benc")
            bdec_s = P.tile([128, ACT_DIM], dt.float32, tag="bdec")
            l2 = P.tile([128, PAY], dt.float32, tag="l2")
            gath = P.tile([128, NCORES, PAY], dt.float32, tag="gath")
            cscr_a = P.tile([128, 2 * SEGW], dt.float32, tag="cscr_a")
            cscr_b = P.tile([128, NCORES, L2_W], dt.float32, tag="cscr_b")
            chp_cols = P.tile([128, NSEG], dt.float32, tag="chp_cols")
            clp_cols = P.tile([128, NSEG], dt.float32, tag="clp_cols")
            ones_t = P.tile([128, 128], dt.float32, tag="ones_t")
            g_in = D.tile([128, PAY], dt.float32)
            g_out = D.tile([NCORES, 128, PAY], dt.float32, addr_space="Shared")

            # scalar state tiles [128, 1]
            def sc(tag):
                return P.tile([128, 1], dt.float32, tag=tag, name=tag)

            sig = sc("sig"); lo0 = sc("lo0"); hi0 = sc("hi0")
            lo = sc("lo"); hi = sc("hi"); clo = sc("clo"); chi = sc("chi")
            chg = sc("chg"); t = sc("t"); ct = sc("ct"); cp = sc("cp")
            pred = sc("pred"); npred = sc("npred")
            inv = sc("inv"); tmp1 = sc("tmp1"); tmp2 = sc("tmp2"); tmp3 = sc("tmp3")

            nc.sync.dma_start(out=xh_s[:], in_=xt_hi[:])
            nc.sync.dma_start(out=xl_s[:], in_=xt_lo[:])
            nc.sync.dma_start(out=benc_s[:], in_=benc[:])
            nc.sync.dma_start(out=bdec_s[:], in_=bdec_b[:])
            nc.vector.memset(ones_t[:], 1.0)
            nc.vector.memset(l2[:, L2_W + 4:PAY], 0.0)

            tt = nc.vector.tensor_tensor
            ts = nc.vector.tensor_scalar
            stt = nc.vector.scalar_tensor_tensor

            with tc.tile_pool(name="rpsum", bufs=2, space="PSUM") as RP:

                def psum_reduce(in_ap, out_ap, scale=None, add_ap=None):
                    # cross-partition sum+broadcast via PE: out[p] = sum_q in[q]
                    rps = RP.tile([128, 1], dt.float32, tag="rps", name="rps")
                    nc.tensor.matmul(rps[:], ones_t[:], in_ap, start=True, stop=True)
                    if scale is not None:
                        nc.vector.tensor_scalar_mul(out_ap, rps[:], scale)
                    elif add_ap is not None:
                        nc.vector.tensor_add(out_ap, rps[:], add_ap)
                    else:
                        nc.vector.tensor_copy(out_ap, rps[:])

                def seg_compact(s):
                    # exact counts vs bracket, band filter, top-8 compaction
                    seg = l1[:, s * SEGW:(s + 1) * SEGW]
                    ts(cscr_a[:, 0:SEGW], seg, hi0[:], None,
                       op0=Alu.is_ge, op1=Alu.add, accum_out=chp_cols[:, s:s + 1])
                    ts(cscr_a[:, SEGW:2 * SEGW], seg, lo0[:], None,
                       op0=Alu.is_ge, op1=Alu.add, accum_out=clp_cols[:, s:s + 1])
                    stt(seg, seg, hi0[:], seg, op0=Alu.is_lt, op1=Alu.mult)
                    nc.vector.max(out=l2[:, s * 8:s * 8 + 8], in_=seg)

                # ================= encode =================
                with (
                    tc.tile_pool(name="wenc", bufs=4) as WP,
                    tc.tile_pool(name="epsum", bufs=4, space="PSUM") as EP,
                ):
                    for ft in range(FT):
                        weh = WP.tile([128, DT, 128], dt.bfloat16, tag="weh")
                        nc.sync.dma_start(out=weh[:], in_=wenc_hi[ft])
                        wel = WP.tile([128, DT, 128], dt.bfloat16, tag="wel")
                        nc.sync.dma_start(out=wel[:], in_=wenc_lo[ft])
                        pairs = ((weh, xh_s), (weh, xl_s), (wel, xh_s))
                        ps = EP.tile([128, ROWS], dt.float32, tag="eps")
                        n_mm = len(pairs) * DT
                        i = 0
                        for dtile in range(DT):
                            for wop, xop in pairs:
                                nc.tensor.matmul(
                                    ps[:],
                                    wop[:, dtile, :],
                                    xop[:, dtile, :],
                                    start=(i == 0),
                                    stop=(i == n_mm - 1),
                                )
                                i += 1
                        pslice = post[:, ft * ROWS:(ft + 1) * ROWS]
                        nc.scalar.activation(
                            out=pslice,
                            in_=ps[:],
                            func=mybir.ActivationFunctionType.Relu,
                            bias=benc_s[:, ft:ft + 1],
                            scale=1.0,
                            accum_out=sums[:, ft:ft + 1],
                        )
                        # L1 candidates: top-8 of each dictionary row
                        nc.vector.max(out=l1[:, ft * 8:ft * 8 + 8], in_=pslice)

                        if ft == SIGT - 1:
                            # provisional sigma -> bracket [lo0, hi0]
                            nc.vector.tensor_reduce(
                                out=tmp1[:], in_=sums[:, 0:SIGT],
                                axis=mybir.AxisListType.X, op=Alu.add)
                            psum_reduce(tmp1[:], sig[:], scale=SIG_SCALE)
                            nc.vector.tensor_scalar_mul(lo0[:], sig[:], A_LO)
                            nc.vector.tensor_scalar_mul(hi0[:], sig[:], A_HI)
                            for s in range(SIGT // 8):
                                seg_compact(s)
                        elif ft >= SIGT and (ft + 1) % 8 == 0:
                            seg_compact((ft + 1) // 8 - 1)

                # prefetch decode weights; the DMAs drain during the topk phase
                with (
                    tc.tile_pool(name="wdec", bufs=NPREF + 2) as WD,
                    tc.tile_pool(name="dpsum", bufs=2, space="PSUM") as DP,
                    tc.tile_pool(name="msk", bufs=10) as MS,
                    tc.tile_pool(name="outs", bufs=2) as OS,
                ):
                    wd_tiles = {}
                    for ft in range(NPREF):
                        wd = WD.tile([128, ACT_DIM], dt.bfloat16, tag="wd")
                        nc.sync.dma_start(out=wd[:], in_=wdect[ft])
                        wd_tiles[ft] = wd

                    # ================= threshold =================
                    nc.vector.tensor_reduce(out=tmp1[:], in_=chp_cols[:],
                                            axis=mybir.AxisListType.X, op=Alu.add)
                    psum_reduce(tmp1[:], tmp2[:])
                    nc.vector.tensor_copy(l2[:, L2_W + 2:L2_W + 3], tmp2[:])
                    nc.vector.tensor_reduce(out=tmp1[:], in_=clp_cols[:],
                                            axis=mybir.AxisListType.X, op=Alu.add)
                    psum_reduce(tmp1[:], tmp3[:])
                    nc.vector.tensor_copy(l2[:, L2_W + 3:L2_W + 4], tmp3[:])
                    nc.vector.tensor_copy(l2[:, L2_W:L2_W + 1], lo0[:])
                    nc.vector.tensor_copy(l2[:, L2_W + 1:L2_W + 2], hi0[:])

                    # AllGather candidates
                    nc.sync.dma_start(out=g_in[:], in_=l2[:])
                    nc.gpsimd.collective_compute(
                        "AllGather",
                        Alu.bypass,
                        replica_groups=[list(range(NCORES))],
                        ins=[g_in.opt()],
                        outs=[g_out.opt()],
                    )
                    for c in range(NCORES):
                        nc.sync.dma_start(out=gath[:, c, :], in_=g_out[c])

                    gv = gath[:, :, 0:L2_W]

                    # global bracket / counts from sidecars
                    nc.vector.tensor_reduce(out=lo[:], in_=gath[:, :, L2_W:L2_W + 1],
                                            axis=mybir.AxisListType.XY, op=Alu.max)
                    nc.vector.tensor_reduce(out=hi[:], in_=gath[:, :, L2_W + 1:L2_W + 2],
                                            axis=mybir.AxisListType.XY, op=Alu.min)
                    nc.vector.tensor_reduce(out=chg[:], in_=gath[:, :, L2_W + 2:L2_W + 3],
                                            axis=mybir.AxisListType.XY, op=Alu.add)
                    nc.vector.tensor_reduce(out=clo[:], in_=gath[:, :, L2_W + 3:L2_W + 4],
                                            axis=mybir.AxisListType.XY, op=Alu.add)

                    def count_ge(t_ap, out_ap):
                        ts(cscr_b[:], gv, t_ap, None,
                           op0=Alu.is_ge, op1=Alu.add, accum_out=cp[:])
                        psum_reduce(cp[:], out_ap, add_ap=chg[:])

                    # exact count at the hi bracket; clo stays the (approximate)
                    # sidecar sum -- it only steers the first interpolation
                    count_ge(hi[:], chi[:])

                    # branch-free false position; the hi side converges onto the
                    # target count from below and is the final threshold
                    for it in range(NSECANT):
                        tt(tmp1[:], clo[:], chi[:], op=Alu.subtract)
                        nc.vector.tensor_scalar_max(tmp1[:], tmp1[:], 1.0)
                        nc.vector.reciprocal(inv[:], tmp1[:])
                        ts(tmp2[:], chi[:], -1.0, CTARGET, op0=Alu.mult, op1=Alu.add)
                        tt(tmp2[:], tmp2[:], inv[:], op=Alu.mult)
                        tt(tmp3[:], lo[:], hi[:], op=Alu.subtract)
                        stt(t[:], tmp3[:], tmp2[:], hi[:], op0=Alu.mult, op1=Alu.add)
                        count_ge(t[:], ct[:])
                        ts(pred[:], ct[:], CTARGET, None, op0=Alu.is_ge)
                        ts(npred[:], pred[:], -1.0, 1.0, op0=Alu.mult, op1=Alu.add)
                        tt(tmp1[:], t[:], lo[:], op=Alu.subtract)
                        stt(lo[:], tmp1[:], pred[:], lo[:], op0=Alu.mult, op1=Alu.add)
                        tt(tmp1[:], ct[:], clo[:], op=Alu.subtract)
                        stt(clo[:], tmp1[:], pred[:], clo[:], op0=Alu.mult, op1=Alu.add)
                        tt(tmp1[:], t[:], hi[:], op=Alu.subtract)
                        stt(hi[:], tmp1[:], npred[:], hi[:], op0=Alu.mult, op1=Alu.add)
                        tt(tmp1[:], ct[:], chi[:], op=Alu.subtract)
                        stt(chi[:], tmp1[:], npred[:], chi[:], op0=Alu.mult, op1=Alu.add)

                    # ================= decode =================
                    HA = ACT_DIM // 2  # 384 -- one matmul per PSUM bank
                    pso = [
                        DP.tile([128, 2, 512], dt.float32, tag="dps", name=f"dps{mt}")
                        for mt in range(MT)
                    ]
                    for ft in range(FT):
                        pslice = post[:, ft * ROWS:(ft + 1) * ROWS]
                        mskt = MS.tile([128, ROWS], dt.bfloat16, tag="mskt")
                        stt(mskt[:], pslice, hi[:], pslice, op0=Alu.is_ge, op1=Alu.mult)
                        wd = wd_tiles.pop(ft, None)
                        if wd is None:
                            wd = WD.tile([128, ACT_DIM], dt.bfloat16, tag="wd")
                            nc.sync.dma_start(out=wd[:], in_=wdect[ft])
                        for mt in range(MT):
                            for h in range(2):
                                nc.tensor.matmul(
                                    pso[mt][:, h, 0:HA],
                                    mskt[:, mt * 128:(mt + 1) * 128],
                                    wd[:, h * HA:(h + 1) * HA],
                                    start=(ft == 0),
                                    stop=(ft == FT - 1),
                                )
                    for mt in range(MT):
                        outs = OS.tile([128, ACT_DIM], dt.float32, tag="outs")
                        for h in range(2):
                            nc.vector.tensor_add(
                                outs[:, h * HA:(h + 1) * HA],
                                pso[mt][:, h, 0:HA],
                                bdec_s[:, h * HA:(h + 1) * HA],
                            )
                        nc.sync.dma_start(out=xhat[mt * 128:(mt + 1) * 128, :], in_=outs[:])

    nc.finalize()
    return nc


def _prep_inputs(x, W_enc, b_enc, W_dec, b_dec):
    import ml_dtypes
    bf16 = ml_dtypes.bfloat16

    x0T = np.ascontiguousarray(
        (x.astype(np.float32) - b_dec.astype(np.float32)[None, :]).T
    )  # [768, 2048]
    WT = np.ascontiguousarray(W_enc.astype(np.float32).T)  # [768, 16384]

    def wlay(a):  # [768, 16384] -> [FT, 128(p=d), DT, 128(f)]
        return np.ascontiguousarray(
            a.reshape(DT, 128, FT, 128).transpose(2, 1, 0, 3)
        )

    xh = x0T.astype(bf16)
    xl = (x0T - xh.astype(np.float32)).astype(bf16)
    Wh = WT.astype(bf16)
    Wl = (WT - Wh.astype(np.float32)).astype(bf16)
    WhL, WlL = wlay(Wh), wlay(Wl)
    WdT = np.ascontiguousarray(W_dec.astype(np.float32).T).astype(bf16).reshape(FT, 128, ACT_DIM)
    bencL = np.ascontiguousarray(b_enc.astype(np.float32).reshape(FT, 128).T)
    bdecB = np.ascontiguousarray(
        np.broadcast_to(b_dec.astype(np.float32)[None, :], (128, ACT_DIM))
    )

    in_maps = []
    for c in range(NCORES):
        sl = slice(c * ROWS, (c + 1) * ROWS)
        m = {
            "xt_hi": np.ascontiguousarray(xh[:, sl].reshape(DT, 128, ROWS).transpose(1, 0, 2)),
            "xt_lo": np.ascontiguousarray(xl[:, sl].reshape(DT, 128, ROWS).transpose(1, 0, 2)),
            "wenc_hi": WhL,
            "wenc_lo": WlL,
            "wdect": WdT,
            "benc": bencL,
            "bdec_b": bdecB,
        }
        in_maps.append(m)
    return in_maps


def _ensure_axon_hooks_shim():
    """concourse's trace path imports antenv.axon_hooks, which some images
    lack; install an equivalent module so tracing degrades (or works, when
    the ctypes hook is available) instead of crashing."""
    import sys, types
    try:
        import antenv.axon_hooks  # noqa: F401
        return
    except ImportError:
        pass
    m = types.ModuleType("antenv.axon_hooks")
    state = {"hook": None}
    m.set_axon_ntff_profile_hook = lambda h: state.__setitem__("hook", h)
    m.get_axon_ntff_profile_hook = lambda: state["hook"]
    sys.modules["antenv.axon_hooks"] = m
    try:
        from trn_agent_boot.trn_boot import _ntff_profile_via_ctypes
        hook = _ntff_profile_via_ctypes("/opt/axon/libaxon_pjrt.so")
        if hook is not None:
            m.set_axon_ntff_profile_hook(hook)
    except Exception:
        pass


def kernel(x, W_enc, b_enc, W_dec, b_dec):
    import os
    _ensure_axon_hooks_shim()
    from concourse import bass_utils
    from concourse.bass_utils import run_bass_kernel_spmd

    in_maps = _prep_inputs(x, W_enc, b_enc, W_dec, b_dec)
    nc = build_nc()
    res = None
    if os.environ.get("KERNEL_TRACE"):
        bass_utils.upload_artifacts = lambda d: ""  # no artifact bucket here
        try:
            res = run_bass_kernel_spmd(nc, in_maps, list(range(NCORES)), trace=True)
        except Exception as e:
            print(f"traced run failed ({type(e).__name__}: {e}); retrying untraced")
            res = None
    if res is None:
        res = run_bass_kernel_spmd(nc, in_maps, list(range(NCORES)))
    if res.exec_time_ns is not None:
        print(f"HW exec time: {res.exec_time_ns} ns")
    out = np.concatenate(
        [np.asarray(res.results[c]["xhat"], dtype=np.float32) for c in range(NCORES)],
        axis=0,
    )
    return out
